# revision 39
# baseline (speedup 1.0000x reference)
"""Trainium2 Bass kernel v4: GQA attention + RoPE + block-diagonal causal
masking, sharded over 8 NeuronCores by KV head group (4 Q heads + 1 KV head
per core, both batches).

v4 design (vs v3):
  - head-major (j, q) attention layout: qr is (64, 4, S); scores use a 2D
    moving AP; sub-diagonal PV accumulates per-head into contiguous psum
    slices (diag tile issued first so accumulate-vs-overwrite stays uniform
    per matmul).
  - softmax reciprocal via the single-instruction DVE approx (the full
    iterative divide was 3.4us per unit and serialized the pipeline).
  - RoPE batched over the whole sequence: projections stage into persistent
    (128, S) tiles; rope is ~13 full-width contiguous DVE ops per batch.
  - macro order: b0 projections -> b1 projections on PE (b0 rope on DVE
    under b1's projections; b1 rope issued per-group as its last chunk is
    staged), then all 32 attention units in a 3-stage pipeline, stepped
    C(i-2), A(i), B(i-1) so psum-bank reuse never blocks the PE queue.
  - PSUM banks: 2 proj/y, 4 scores/o, 2 y — o_proj gets 4 distinct banks
    per unit (no copy-wait ping-pong).
  - per-unit diag mask bias tiles built on-the-fly on GpSimd in a ring.
"""
import sys
sys.path.insert(0, "/opt/trn_rl_repo")
import numpy as np

B, S, DIM = 2, 2048, 2048
NH, NKV, HD = 32, 8, 64
HPC = NH // 8            # 4 q-heads per core
MLOC = HPC * HD          # 256 local q dims
TQ = 128                 # attention query tile
NCORES = 8
NKC = DIM // 128         # 16 contraction chunks
NTC = S // 128           # 16 token tiles
SCALE = 1.0 / 8.0
NEG = -30000.0

_nc_cache = {}
DEBUG_DUMP = False


def _schedule(doc_ids):
    """Per (batch, query-tile): band start tile t0, sub-diagonal column limit
    c1, band-start partition offset off1, and diagonal boundary offsets."""
    doc = np.asarray(doc_ids)
    sched = []
    for b in range(B):
        d = doc[b]
        change = np.empty(S, dtype=np.int64)
        change[0] = 0
        idx = np.arange(1, S)
        change[1:] = np.where(d[1:] != d[:-1], idx, 0)
        start_idx = np.maximum.accumulate(change)
        bounds = np.nonzero(change)[0]
        per_qc = []
        for qc in range(S // TQ):
            q0 = qc * TQ
            s1 = int(start_idx[q0])
            t0 = s1 // 128
            inner = [int(p) - q0 for p in bounds if q0 < p < q0 + TQ]
            c1 = inner[0] if inner else TQ
            per_qc.append({"t0": t0, "c1": c1, "off1": s1 - t0 * 128,
                           "bnds": tuple(inner)})
        sched.append(per_qc)
    return sched


def _build_nc(sched):
    import concourse.bacc as bacc
    import concourse.mybir as mybir
    import concourse.tile as tile
    from concourse.masks import make_identity

    F32, BF16 = mybir.dt.float32, mybir.dt.bfloat16
    Exp = mybir.ActivationFunctionType.Exp
    GE = mybir.AluOpType.is_ge

    nc = bacc.Bacc()
    xT = nc.dram_tensor("xT", (B, DIM, S), BF16, kind="ExternalInput")
    wq = nc.dram_tensor("wq", (DIM, MLOC), BF16, kind="ExternalInput")
    wkv = nc.dram_tensor("wkv", (DIM, 128), BF16, kind="ExternalInput")
    wo = nc.dram_tensor("wo", (MLOC, DIM), BF16, kind="ExternalInput")
    cos128 = nc.dram_tensor("cos128", (128, S), BF16, kind="ExternalInput")
    sin128 = nc.dram_tensor("sin128", (128, S), BF16, kind="ExternalInput")
    y = nc.dram_tensor("y", (B, S, DIM), BF16, kind="ExternalOutput")

    # band-start exp bias columns, one per distinct off1 value
    offs = sorted({e["off1"] for sb in sched for e in sb if e["off1"] > 0})
    off_col = {o: i for i, o in enumerate(offs)}
    units = [(b, qc) for b in range(B) for qc in range(S // TQ)]
    NU = len(units)

    with tile.TileContext(nc) as tc:
        with (
            tc.tile_pool(name="const", bufs=1) as cst,
            tc.tile_pool(name="xt", bufs=3) as xtp,
            tc.tile_pool(name="big", bufs=2) as big,
            tc.tile_pool(name="rope", bufs=1) as rp,
            tc.tile_pool(name="dbr", bufs=3) as dbp,
            tc.tile_pool(name="pt", bufs=7) as ptp,
            tc.tile_pool(name="small", bufs=3) as sp,
            tc.tile_pool(name="ysb", bufs=3) as yp,
            tc.tile_pool(name="pa", bufs=4, space="PSUM") as pa,
        ):
            # ---- weights / tables (SP seq); split so the first proj
            # matmuls start after ~64KB instead of the full 1MB, and keep
            # wo (not needed until the first o_proj) off the early BW.
            # warm the PE HAM clock-gate during the startup DMA window
            dum = cst.tile([64, 64], BF16)
            nc.vector.memset(dum[:], 0.5)
            wu_ps = pa.tile([128, 64], F32, tag="pa", name="wu_ps")
            for _w in range(70):
                nc.tensor.matmul(wu_ps[0:64, :], dum[:], dum[:],
                                 start=True, stop=True)
            wq_sb = cst.tile([128, NKC, MLOC], BF16)
            wq_r = wq[:].rearrange("(kc p) m -> p kc m", p=128)
            wkv_sb = cst.tile([128, NKC, 128], BF16)
            wkv_r = wkv[:].rearrange("(kc p) m -> p kc m", p=128)
            for lo, hi in ((0, 2), (2, 6), (6, NKC)):
                nc.sync.dma_start(wq_sb[:, lo:hi, :], wq_r[:, lo:hi, :])
            for lo, hi in ((0, 2), (2, 6), (6, NKC)):
                nc.sync.dma_start(wkv_sb[:, lo:hi, :], wkv_r[:, lo:hi, :])
            cos_sb = cst.tile([128, S], BF16)
            nc.sync.dma_start(cos_sb[:], cos128[:])
            sin_sb = cst.tile([128, S], BF16)
            nc.sync.dma_start(sin_sb[:], sin128[:])
            wo_sb = cst.tile([128, 2, DIM], BF16)   # loaded after b0's proj
            ident = cst.tile([64, 64], BF16)
            make_identity(nc, ident[:])
            id128 = cst.tile([128, 128], BF16)
            make_identity(nc, id128[:])

            # ---- x chunk prefetch machinery (Pool seq) ----
            chunks = [(b, tqi) for b in range(B) for tqi in range(4)]
            xt_tiles = {}

            def issue_xt(gi):
                bb, ti = chunks[gi]
                t = xtp.tile([128, NKC, 512], BF16, tag="xt", name=f"xt{bb}{ti}")
                xt_tiles[(bb, ti)] = t
                src = xT[bb].rearrange("(kc p) s -> p kc s", p=128)
                tsl = slice(ti * 512, ti * 512 + 512)
                if gi == 0:
                    for lo, hi in ((0, 2), (2, 6), (6, 11), (11, NKC)):
                        nc.gpsimd.dma_start(t[:, lo:hi, :], src[:, lo:hi, tsl])
                else:
                    nc.gpsimd.dma_start(t[:], src[:, :, tsl])

            issue_xt(0)
            issue_xt(1)

            # ---- mask bias tiles, head-major (128, 4, TQ) ----
            causal_b = cst.tile([128, 4, TQ], BF16)

            def build_causal():
                nc.gpsimd.memset(causal_b[:], 0.0)
                nc.gpsimd.affine_select(
                    out=causal_b[:], in_=causal_b[:], compare_op=GE, fill=NEG,
                    base=0, pattern=[[0, 4], [1, TQ]], channel_multiplier=-1)

            diag_b = {}

            def build_diag_bias(b, qc):
                # ring-buffered per-unit bias tile (causal + doc boundaries)
                bnds = sched[b][qc]["bnds"]
                if not bnds:
                    return
                t = dbp.tile([128, 4, TQ], BF16, tag="db", name=f"db{b}{qc}")
                nc.gpsimd.tensor_copy(t[:], causal_b[:])
                for p in bnds:
                    nc.gpsimd.affine_select(
                        out=t[:, :, p:TQ], in_=t[:, :, p:TQ],
                        compare_op=GE, fill=NEG, base=-p,
                        pattern=[[0, 4], [0, TQ - p]], channel_multiplier=1)
                diag_b[(b, qc)] = t

            bandb = None
            if offs:
                bandb = cst.tile([128, len(offs)], F32)

            def build_bandb():
                nc.gpsimd.memset(bandb[:], 0.0)
                for o, i in off_col.items():
                    nc.gpsimd.affine_select(
                        out=bandb[:, i:i + 1], in_=bandb[:, i:i + 1],
                        compare_op=GE, fill=NEG, base=-o,
                        pattern=[[0, 1]], channel_multiplier=1)

            # ---- persistent per-batch tiles ----
            q_st, kv_st, qr_d, krt_d, vaug_d, or2t_d = {}, {}, {}, {}, {}, {}
            for b in range(B):
                q_st[b] = [big.tile([128, S], BF16, tag=f"qst{g}", name=f"qst{b}{g}")
                           for g in range(2)]
                kv_st[b] = big.tile([128, S], BF16, tag="kvst", name=f"kvst{b}")
                qr_d[b] = big.tile([64, 4, S], BF16, tag="qr", name=f"qr{b}")
                krt_d[b] = big.tile([64, S], BF16, tag="krt", name=f"krt{b}")
                vaug_d[b] = big.tile([128, NTC, 128], BF16, tag="vaug",
                                     name=f"vaug{b}")
                or2t_d[b] = big.tile([128, 2, S], BF16, tag="or2t",
                                     name=f"or2t{b}")
                nc.vector.memset(vaug_d[b][:, :, 64:128], 1.0)

            tp_n = [0]

            def flush_tp(bb, ti):
                kvs = kv_st[bb]
                for tc4 in range(4):
                    tp_n[0] += 1
                    ptr = pa.tile([128, 64], BF16, tag="pa",
                                  name=f"ptr{tp_n[0]}")
                    c0 = ti * 512 + tc4 * 128
                    nc.tensor.transpose(
                        ptr[:], kvs[0:64, c0:c0 + 128], ident[:])
                    nc.scalar.copy(vaug_d[bb][:, ti * 4 + tc4, 0:64], ptr[:])

            def rope_tqi(b, tqi):
                # per-chunk rope (512 tokens): spreads DVE work through the
                # projection window instead of a post-proj burst
                tsl = slice(tqi * 512, tqi * 512 + 512)
                qr = qr_d[b]
                for g in range(2):
                    xs = q_st[b][g]
                    u = rp.tile([128, 512], BF16, tag="rc")
                    for blk in (0, 64):
                        nc.vector.tensor_mul(
                            u[blk:blk + 32], xs[blk + 32:blk + 64, tsl],
                            sin_sb[blk + 32:blk + 64, tsl])
                        nc.vector.tensor_mul(
                            u[blk + 32:blk + 64], xs[blk:blk + 32, tsl],
                            sin_sb[blk:blk + 32, tsl])
                    t2 = rp.tile([128, 512], BF16, tag="rd")
                    nc.vector.tensor_mul(t2[:], xs[:, tsl], cos_sb[:, tsl])
                    nc.vector.tensor_add(qr[0:64, 2 * g, tsl], u[0:64], t2[0:64])
                    nc.vector.tensor_add(
                        qr[0:64, 2 * g + 1, tsl], u[64:128], t2[64:128])
                kvs = kv_st[b]
                u = rp.tile([128, 512], BF16, tag="rc")
                nc.vector.tensor_mul(u[64:96], kvs[96:128, tsl],
                                     sin_sb[96:128, tsl])
                nc.vector.tensor_mul(u[96:128], kvs[64:96, tsl],
                                     sin_sb[64:96, tsl])
                t2 = rp.tile([128, 512], BF16, tag="rd")
                nc.vector.tensor_mul(t2[64:128], kvs[64:128, tsl],
                                     cos_sb[64:128, tsl])
                nc.vector.tensor_add(krt_d[b][:, tsl], u[64:128], t2[64:128])

            # ================= projections (PE+Act) =================
            def proj_batch(b):
                for tqi in range(4):
                    gi = b * 4 + tqi
                    if gi + 2 < len(chunks):
                        issue_xt(gi + 2)
                    t0c = tqi * 512
                    tsl = slice(t0c, t0c + 512)
                    xt_sb = xt_tiles[(b, tqi)]
                    for what in ("q0", "q1", "kv"):
                        gps = pa.tile([128, 512], F32, tag="py", bufs=2,
                                      name=f"gps{b}{tqi}{what}")
                        for kc in range(NKC):
                            if what == "q0":
                                lhs = wq_sb[:, kc, 0:128]
                            elif what == "q1":
                                lhs = wq_sb[:, kc, 128:256]
                            else:
                                lhs = wkv_sb[:, kc, :]
                            nc.tensor.matmul(
                                gps[:], lhs, xt_sb[:, kc, :],
                                start=(kc == 0), stop=(kc == NKC - 1))
                        if what == "kv":
                            nc.scalar.copy(kv_st[b][:, tsl], gps[:])
                            flush_tp(b, tqi)
                        else:
                            g = 0 if what == "q0" else 1
                            nc.scalar.copy(q_st[b][g][:, tsl], gps[:])
                    if b == 0:
                        rope_tqi(0, tqi)
                        if tqi == 0:
                            build_causal()
                            if bandb is not None:
                                build_bandb()
                            bias_to(4)
                    else:
                        rope_tqi(1, tqi)
                        if tqi == 3:
                            pipe_to(2)

            # ================= attention pipeline =================
            st = {}

            def stage_a(b, qc):
                qr, krt = qr_d[b], krt_d[b]
                q0 = qc * TQ
                ent = sched[b][qc]
                t0, c1, off1 = ent["t0"], ent["c1"], ent["off1"]
                pts = []
                # diagonal tile first (full width, start=True)
                s_ps = pa.tile([128, 4 * TQ], F32, tag="pa",
                               name=f"spsd{b}{qc}")
                nc.tensor.matmul(
                    s_ps[:], krt[:, qc * 128:(qc + 1) * 128],
                    qr[:, :, q0:q0 + TQ], start=True, stop=False)
                db = diag_b.pop((b, qc), None)
                nc.tensor.matmul(
                    s_ps[:], id128[:],
                    (db if db is not None else causal_b)[:],
                    start=False, stop=True)
                pt = ptp.tile([128, 4 * TQ], BF16, tag="pt")
                bias = 0.0
                if t0 == qc and off1 > 0:
                    bias = bandb[:, off_col[off1]:off_col[off1] + 1]
                nc.scalar.activation(pt[:], s_ps[:], Exp, bias=bias, scale=SCALE)
                if DEBUG_DUMP and (b, qc) == (0, 0):
                    dpt = nc.dram_tensor("dbg_pt00", (128, 4 * TQ), BF16,
                                         kind="ExternalOutput")
                    nc.sync.dma_start(dpt[:], pt[:])
                pts.append((qc, TQ, pt))
                for kt in range(t0, qc):
                    s_ps2 = pa.tile([128, 4 * TQ], F32, tag="pa",
                                    name=f"sps{b}{qc}{kt}")
                    nc.tensor.matmul(
                        s_ps2[:, 0:4 * c1], krt[:, kt * 128:(kt + 1) * 128],
                        qr[:, :, q0:q0 + c1], start=True, stop=True)
                    pt2 = ptp.tile([128, 4 * TQ], BF16, tag="pt")
                    bias2 = 0.0
                    if kt == t0 and off1 > 0:
                        bias2 = bandb[:, off_col[off1]:off_col[off1] + 1]
                    nc.scalar.activation(
                        pt2[:, 0:4 * c1], s_ps2[:, 0:4 * c1], Exp,
                        bias=bias2, scale=SCALE)
                    pts.append((kt, c1, pt2))
                st[(b, qc)] = pts

            def stage_b(b, qc):
                vaug = vaug_d[b]
                pts = st.pop((b, qc))
                nsub = len(pts) - 1
                o_ps = pa.tile([128, 4 * TQ], F32, tag="pa", name=f"ops{b}{qc}")
                ktd, _, pt_d = pts[0]
                nc.tensor.matmul(o_ps[:], vaug[:, ktd, :], pt_d[:],
                                 start=True, stop=(nsub == 0))
                for si, (kt, c1x, pt2) in enumerate(pts[1:]):
                    if c1x == TQ:
                        nc.tensor.matmul(
                            o_ps[:], vaug[:, kt, :], pt2[:],
                            start=False, stop=(si == nsub - 1))
                    else:
                        for j in range(4):
                            nc.tensor.matmul(
                                o_ps[:, j * TQ:j * TQ + c1x], vaug[:, kt, :],
                                pt2[:, j * c1x:(j + 1) * c1x],
                                start=False,
                                stop=(si == nsub - 1 and j == 3))
                # the custom-DVE recip misreads PSUM on HW: stage Z to SBUF
                zs = sp.tile([64, 4 * TQ], F32, tag="zs")
                nc.scalar.copy(zs[:], o_ps[64:128, :])
                zb = sp.tile([64, 4 * TQ], F32, tag="zb")
                nc.vector.reciprocal_approx_fast(zb[:], zs[:])
                if DEBUG_DUMP and (b, qc) == (0, 0):
                    dzb = nc.dram_tensor("dbg_zb00", (64, 4 * TQ), F32,
                                         kind="ExternalOutput")
                    nc.sync.dma_start(dzb[:], zb[:])
                or2 = or2t_d[b]
                q0 = qc * TQ
                o_r = o_ps[:].rearrange("p (hp h q) -> p h hp q", hp=2, h=2)
                zb_r = zb[:].rearrange("p (hp h q) -> p h hp q", hp=2, h=2)
                for h in range(2):
                    nc.vector.tensor_mul(
                        or2[h * 64:(h + 1) * 64, :, q0:q0 + TQ],
                        o_r[0:64, h, :, :], zb_r[:, h, :, :])

            def stage_c(b, qc):
                or2 = or2t_d[b]
                q0 = qc * TQ
                y_row = yp.tile([128, DIM], BF16, tag="ysb")
                y_ps2 = pa.tile([128, 2, 512], F32, tag="py", bufs=2,
                                name=f"yps01{b}{qc}")
                for mc in range(2):
                    for hp in range(2):
                        nc.tensor.matmul(
                            y_ps2[:, mc, :], or2[:, hp, q0:q0 + TQ],
                            wo_sb[:, hp, mc * 512:(mc + 1) * 512],
                            start=(hp == 0), stop=(hp == 1))
                nc.scalar.copy(y_row[:, 0:1024],
                               y_ps2[:].rearrange("p m n -> p (m n)"))
                nc.sync.dma_start(y[b, q0:q0 + TQ, 0:1024], y_row[:, 0:1024])
                y_ps3 = pa.tile([128, 2, 512], F32, tag="py", bufs=2,
                                name=f"yps23{b}{qc}")
                for mc in range(2):
                    for hp in range(2):
                        nc.tensor.matmul(
                            y_ps3[:, mc, :], or2[:, hp, q0:q0 + TQ],
                            wo_sb[:, hp, (mc + 2) * 512:(mc + 3) * 512],
                            start=(hp == 0), stop=(hp == 1))
                nc.vector.tensor_copy(
                    y_row[:, 1024:2048],
                    y_ps3[:].rearrange("p m n -> p (m n)"))
                nc.sync.dma_start(y[b, q0:q0 + TQ, 1024:2048],
                                  y_row[:, 1024:2048])

            # ---- bias-build ring: build unit i's diag bias ~4 steps ahead
            next_bias = [0]

            def bias_to(k):
                while next_bias[0] < min(k, NU):
                    bb, qq = units[next_bias[0]]
                    build_diag_bias(bb, qq)
                    next_bias[0] += 1

            # ================= program order =================
            next_i = [0]

            def pipe_step(i):
                if i < NU:
                    stage_a(*units[i])
                if i >= 2:
                    stage_c(*units[i - 2])
                if 1 <= i < NU + 1:
                    stage_b(*units[i - 1])
                bias_to(i + 5)

            def pipe_to(k):
                while next_i[0] < min(k, NU + 2):
                    pipe_step(next_i[0])
                    next_i[0] += 1

            proj_batch(0)
            nc.sync.dma_start(wo_sb[:],
                              wo[:].rearrange("(c p) m -> p c m", p=128))
            proj_batch(1)
            pipe_to(NU + 2)

            if DEBUG_DUMP:
                dq = nc.dram_tensor("dbg_qr", (64, 4, S), BF16,
                                    kind="ExternalOutput")
                nc.sync.dma_start(dq[:], qr_d[0][:])
                dk = nc.dram_tensor("dbg_krt", (64, S), BF16,
                                    kind="ExternalOutput")
                nc.sync.dma_start(dk[:], krt_d[0][:])
                dv = nc.dram_tensor("dbg_vaug", (128, NTC, 128), BF16,
                                    kind="ExternalOutput")
                nc.sync.dma_start(dv[:], vaug_d[0][:])
                do = nc.dram_tensor("dbg_or", (2, 128, S), BF16,
                                    kind="ExternalOutput")
                for m in range(2):
                    nc.sync.dma_start(do[m], or2t_d[0][:, m, :])
                dqs = nc.dram_tensor("dbg_qst", (2, 128, S), BF16,
                                     kind="ExternalOutput")
                for g in range(2):
                    nc.sync.dma_start(dqs[g], q_st[0][g][:])
                dkv = nc.dram_tensor("dbg_kvst", (128, S), BF16,
                                     kind="ExternalOutput")
                nc.sync.dma_start(dkv[:], kv_st[0][:])

    nc.finalize()
    return nc


def _prep_inputs(x, rope_cos, rope_sin, doc_ids, Wq, Wk, Wv, Wo):
    import ml_dtypes
    BF = ml_dtypes.bfloat16
    x = np.asarray(x, np.float32)
    xT = np.ascontiguousarray(x.transpose(0, 2, 1)).astype(BF)
    cosT = np.asarray(rope_cos, np.float32).T          # (32, S)
    sinT = np.asarray(rope_sin, np.float32).T
    cos128 = np.tile(np.concatenate([cosT, cosT], 0), (2, 1)).astype(BF)
    sin128 = np.tile(np.concatenate([sinT, -sinT], 0), (2, 1)).astype(BF)
    sched = _schedule(doc_ids)
    Wq = np.asarray(Wq, np.float32)
    Wk = np.asarray(Wk, np.float32)
    Wv = np.asarray(Wv, np.float32)
    Wo = np.asarray(Wo, np.float32)
    in_maps = []
    for c in range(NCORES):
        wq_c = np.ascontiguousarray(Wq[c * MLOC:(c + 1) * MLOC].T).astype(BF)
        wk_c = Wk[c * HD:(c + 1) * HD].T
        wv_c = Wv[c * HD:(c + 1) * HD].T
        wkv_c = np.ascontiguousarray(np.concatenate([wv_c, wk_c], 1)).astype(BF)
        wo_c = np.ascontiguousarray(Wo[:, c * MLOC:(c + 1) * MLOC].T).astype(BF)
        in_maps.append({
            "xT": xT, "wq": wq_c, "wkv": wkv_c, "wo": wo_c,
            "cos128": cos128, "sin128": sin128,
        })
    return sched, in_maps


def _sched_key(sched):
    return tuple(
        tuple((e["t0"], e["c1"], e["off1"], e["bnds"]) for e in sb)
        for sb in sched)


def kernel(x, rope_cos, rope_sin, doc_ids, Wq, Wk, Wv, Wo):
    from concourse.bass_utils import run_bass_kernel_spmd
    sched, in_maps = _prep_inputs(
        x, rope_cos, rope_sin, doc_ids, Wq, Wk, Wv, Wo)
    key = _sched_key(sched)
    nc = _nc_cache.get(key)
    if nc is None:
        nc = _build_nc(sched)
        _nc_cache[key] = nc
    res = run_bass_kernel_spmd(nc, in_maps, core_ids=list(range(NCORES)))
    y = np.zeros((B, S, DIM), np.float32)
    for c in range(NCORES):
        y += res.results[c]["y"].astype(np.float32)
    return y
